# revision 3
# baseline (speedup 1.0000x reference)
"""Trainium2 Bass kernel for a 2-layer LSTM (B=512, T=1024, D=128, H=256, OUT=1).

Strategy: data-parallel over batch (8 cores x 64 rows). Each core runs the full
T=1024 recurrence on its batch shard. On-chip layout is "transposed": partition
dim = feature chunk (128 wide), free dim = 64*chunk_idx + batch, so h-state
tiles are directly the moving (rhs) operand of the recurrent matmuls.

v2 changes vs v1:
- One PSUM bank [128, 512] per layer per parity holds ALL 8 gate chunks in
  order [f0 f1 i0 i1 o0 o1 g0 g1]; the g-gate weight columns are pre-scaled
  x2 on host so a SINGLE 512-wide sigmoid computes sigmoid for f,i,o and
  s = sigmoid(2*ghat) for g (tanh(x) = 2*sigmoid(2x) - 1). A one-op DVE
  tensor_scalar affine (2s - 1) recovers tanh(g). This cuts the Scalar (ACT)
  engine from 6 to 4 instructions per step - ACT was the bottleneck engine
  (83% busy) in v1.
- PE queue reordered per iteration: [xproj(t+2) | l2-h2part(t) | l1-rec(t+1) |
  l2-h1part(t)] so always-ready matmuls fill the PE idle window while waiting
  for h1, absorbing the ~190ns post-idle pipeline-restart latency that
  previously landed on the recurrence critical path.
"""

import numpy as np
import ml_dtypes

B, T, D = 512, 1024, 128
H = 256
NCORES = 8
BL = B // NCORES  # 64 batch rows per core
XBLK = 16  # timesteps per x DMA block
# gate chunk permutation: original 4H chunk order is f(0,1) i(2,3) g(4,5) o(6,7);
# on-chip order is [f0 f1 i0 i1 o0 o1 g0 g1] (figo then g).
PERM = [0, 1, 2, 3, 6, 7, 4, 5]
G_POS = (6, 7)  # on-chip chunk positions holding the g gate (weights x2)

_BF16 = np.float16


def _build(t_steps, with_b1, with_b2):
    import concourse.bass as bass  # noqa: F401
    from concourse.tile import add_dep_helper  # noqa: F401
    import concourse.mybir as mybir
    import concourse.tile as tile
    from concourse import bacc

    dt = mybir.dt
    AF = mybir.ActivationFunctionType
    ALU = mybir.AluOpType
    nblk = (t_steps + XBLK - 1) // XBLK

    nc = bacc.Bacc("TRN2", target_bir_lowering=False, debug=False, num_devices=NCORES)
    x_in = nc.declare_dram_parameter(
        "x", [nblk, 128, XBLK, BL], dt.float16, isOutput=False
    )
    w1_in = nc.declare_dram_parameter("w1", [128, 3 * 8 * 128], dt.float16, isOutput=False)
    w2_in = nc.declare_dram_parameter("w2", [128, 4 * 8 * 128], dt.float16, isOutput=False)
    if with_b1:
        b1_in = nc.declare_dram_parameter("b1", [8, 128], dt.float16, isOutput=False)
    if with_b2:
        b2_in = nc.declare_dram_parameter("b2", [8, 128], dt.float16, isOutput=False)
    if with_b1 or with_b2:
        ind_in = nc.declare_dram_parameter("ind", [8, 512], dt.float16, isOutput=False)
    y_out = nc.declare_dram_parameter("y", [128, 128], dt.float32, isOutput=True)

    with tile.TileContext(nc) as tc:
        with (
            tc.tile_pool(name="singles", bufs=1) as singles,
            tc.tile_pool(name="temps", bufs=6) as temps,
            tc.tile_pool(name="psum", bufs=1, space="PSUM") as psum,
        ):
            w1 = singles.tile([128, 3 * 8 * 128], dt.float16)
            w2 = singles.tile([128, 4 * 8 * 128], dt.float16)
            nc.sync.dma_start(out=w1, in_=w1_in[:])
            nc.sync.dma_start(out=w2, in_=w2_in[:])
            if with_b1:
                b1s = singles.tile([8, 128], dt.float16)
                nc.sync.dma_start(out=b1s, in_=b1_in[:])
            if with_b2:
                b2s = singles.tile([8, 128], dt.float16)
                nc.sync.dma_start(out=b2s, in_=b2_in[:])
            if with_b1 or with_b2:
                ind = singles.tile([8, 512], dt.float16)
                nc.sync.dma_start(out=ind, in_=ind_in[:])

            xr = [
                singles.tile([128, XBLK * BL], dt.float16, name=f"xr{i}")
                for i in range(3)
            ]
            h1r = [singles.tile([128, 128], dt.float16, name=f"h1r{i}") for i in range(2)]
            h2r = [singles.tile([128, 128], dt.float16, name=f"h2r{i}") for i in range(2)]
            cg1 = singles.tile([128, 256], dt.float16)  # [c | tanh(g)] co-tile
            cg2 = singles.tile([128, 256], dt.float16)
            out_sb = singles.tile([128, 128], dt.float32)
            for tl in (h1r[0], h1r[1], h2r[0], h2r[1], cg1, cg2):
                nc.gpsimd.memset(tl, 0.0)

            gb1 = [psum.tile([128, 512], dt.float32, name=f"gb1_{i}") for i in range(2)]
            gb2 = [psum.tile([128, 512], dt.float32, name=f"gb2_{i}") for i in range(2)]

            nc.sync.dma_start(out=xr[0], in_=x_in[0])

            mm = nc.tensor.matmul

            def w1_tile(k, j):
                i = (k * 8 + j) * 128
                return w1[:, i : i + 128]

            def w2_tile(k, j):
                i = (k * 8 + j) * 128
                return w2[:, i : i + 128]

            def xs_of(t):
                blk = t // XBLK
                tt = t % XBLK
                return xr[blk % 3][:, tt * BL : (tt + 1) * BL]

            def l1_mm_xproj(t):
                p = t % 2
                blk = t // XBLK
                tt = t % XBLK
                if tt == 0 and blk + 1 < nblk:
                    nc.sync.dma_start(out=xr[(blk + 1) % 3], in_=x_in[blk + 1])
                xs = xs_of(t)
                for j in range(8):
                    mm(gb1[p][:, 64 * j : 64 * j + 64], w1_tile(0, j), xs,
                       start=(j == 0), stop=False, skip_group_check=True)
                if with_b1:
                    mm(gb1[p][:, :], b1s, ind, start=False, stop=False,
                       skip_group_check=True)

            def l1_mm_rec(t):
                p = t % 2
                h1_prev = h1r[(t + 1) % 2]
                for k in (1, 2):
                    hk = h1_prev[:, 64 * (k - 1) : 64 * k]
                    for j in range(8):
                        mm(gb1[p][:, 64 * j : 64 * j + 64], w1_tile(k, j), hk,
                           start=False, stop=(k == 2 and j == 7),
                           skip_group_check=True)

            def l2_mm_h2(t):
                p = t % 2
                h2_prev = h2r[(t + 1) % 2]
                for k in (2, 3):
                    hk = h2_prev[:, 64 * (k - 2) : 64 * (k - 1)]
                    for j in range(8):
                        mm(gb2[p][:, 64 * j : 64 * j + 64], w2_tile(k, j), hk,
                           start=(k == 2 and j == 0), stop=False,
                           skip_group_check=True)
                if with_b2:
                    mm(gb2[p][:, :], b2s, ind, start=False, stop=False,
                       skip_group_check=True)

            def l2_mm_h1(t):
                p = t % 2
                h1_cur = h1r[t % 2]
                for k in (0, 1):
                    hk = h1_cur[:, 64 * k : 64 * (k + 1)]
                    for j in range(8):
                        mm(gb2[p][:, 64 * j : 64 * j + 64], w2_tile(k, j), hk,
                           start=False, stop=(k == 1 and j == 7),
                           skip_group_check=True)

            def l1_ew(t):
                p = t % 2
                figog = temps.tile([128, 512], dt.float16, name="figog1")
                nc.scalar.activation(figog, gb1[p][:, :], AF.Sigmoid)
                # g = 2*sigmoid(2*ghat) - 1 == tanh(ghat)
                nc.vector.tensor_scalar(
                    cg1[:, 128:256], figog[:, 384:512], 2.0, 1.0, ALU.mult,
                    ALU.subtract)
                fcig = temps.tile([128, 256], dt.float16, name="fcig1")
                nc.vector.tensor_mul(fcig, figog[:, 0:256], cg1)
                nc.vector.tensor_add(cg1[:, 0:128], fcig[:, 0:128], fcig[:, 128:256])
                th = temps.tile([128, 128], dt.float16, name="th1")
                nc.scalar.activation(th, cg1[:, 0:128], AF.Tanh)
                nc.vector.tensor_mul(h1r[t % 2][:, 0:64], figog[:, 256:320], th[:, 0:64])
                nc.vector.tensor_mul(h1r[t % 2][:, 64:128], figog[:, 320:384], th[:, 64:128])

            def l2_ew(t):
                p = t % 2
                figog = temps.tile([128, 512], dt.float16, name="figog2")
                nc.scalar.activation(figog, gb2[p][:, :], AF.Sigmoid)
                nc.vector.tensor_scalar(
                    cg2[:, 128:256], figog[:, 384:512], 2.0, 1.0, ALU.mult,
                    ALU.subtract)
                fcig = temps.tile([128, 256], dt.float16, name="fcig2")
                nc.vector.tensor_mul(fcig, figog[:, 0:256], cg2)
                nc.vector.tensor_add(cg2[:, 0:128], fcig[:, 0:128], fcig[:, 128:256])
                th = temps.tile([128, 128], dt.float16, name="th2")
                nc.scalar.activation(th, cg2[:, 0:128], AF.Tanh)
                nc.vector.tensor_mul(h2r[t % 2], figog[:, 256:384], th)
                if t == t_steps - 1:
                    nc.vector.tensor_mul(out_sb, figog[:, 256:384], th)
                    nc.sync.dma_start(out=y_out[:], in_=out_sb)

            # software pipeline: iteration tau advances L1 to step tau+1 and
            # L2 to step tau; xproj runs two steps ahead as always-ready PE
            # filler work.
            l1_mm_xproj(0)
            l1_mm_rec(0)
            if t_steps > 1:
                l1_mm_xproj(1)
            l1_ew(0)
            for tau in range(t_steps):
                if tau + 2 < t_steps:
                    l1_mm_xproj(tau + 2)
                l2_mm_h2(tau)
                if tau + 1 < t_steps:
                    l1_mm_rec(tau + 1)
                l2_mm_h1(tau)
                if tau + 1 < t_steps:
                    l1_ew(tau + 1)
                l2_ew(tau)

    nc.compile()
    return nc


_NC_CACHE = {}


def _get_nc(t_steps, with_b1, with_b2):
    key = (t_steps, with_b1, with_b2)
    if key not in _NC_CACHE:
        _NC_CACHE[key] = _build(t_steps, with_b1, with_b2)
    return _NC_CACHE[key]


def _pack_w(W, kchunks):
    """W [128*kchunks, 1024] -> [128, kchunks*8*128] bf16, PERM chunk order,
    with the g-gate chunk columns scaled x2 (tanh-via-sigmoid)."""
    out = np.empty((128, kchunks, 8, 128), dtype=np.float32)
    for k in range(kchunks):
        for j in range(8):
            m = PERM[j]
            w = W[128 * k : 128 * (k + 1), 128 * m : 128 * (m + 1)]
            if j in G_POS:
                w = w * 2.0
            out[:, k, j, :] = w
    return np.ascontiguousarray(out.reshape(128, kchunks * 8 * 128).astype(_BF16))


def _pack_bias(b):
    """b [1024] -> [8, 128] lhsT rows in PERM order (g rows x2)."""
    bb = np.zeros((8, 128), dtype=np.float32)
    for j in range(8):
        bb[j, :] = b[128 * PERM[j] : 128 * (PERM[j] + 1)]
        if j in G_POS:
            bb[j, :] *= 2.0
    return bb.astype(_BF16)


def _make_ind():
    ind = np.zeros((8, 512), dtype=_BF16)
    for j in range(8):
        ind[j, 64 * j : 64 * (j + 1)] = 1
    return ind


def _pack_x_core(xc, t_steps):
    """xc [BL, T, D] f32 -> [nblk, 128, XBLK, BL] bf16 (partition = d)."""
    nblk = (t_steps + XBLK - 1) // XBLK
    xt = xc.transpose(1, 2, 0)  # [T, D, BL]
    xt = xt.reshape(nblk, XBLK, D, BL).transpose(0, 2, 1, 3)  # [nblk, D, XBLK, BL]
    return np.ascontiguousarray(xt.astype(_BF16))


TRACE = False  # set by test harness to capture a HW profile
LAST_EXEC_NS = None


def kernel(x, W1, b1, W2, b2, Wout, bout):
    global LAST_EXEC_NS
    from concourse.bass_utils import run_bass_kernel_spmd

    x = np.asarray(x)
    W1 = np.asarray(W1, dtype=np.float32)
    b1 = np.asarray(b1, dtype=np.float32)
    W2 = np.asarray(W2, dtype=np.float32)
    b2 = np.asarray(b2, dtype=np.float32)
    Wout = np.asarray(Wout, dtype=np.float32)
    bout = np.asarray(bout, dtype=np.float32)
    t_steps = x.shape[1]

    with_b1 = bool(np.any(b1))
    with_b2 = bool(np.any(b2))
    nc = _get_nc(t_steps, with_b1, with_b2)

    base = {"w1": _pack_w(W1, 3), "w2": _pack_w(W2, 4)}
    if with_b1:
        base["b1"] = _pack_bias(b1)
    if with_b2:
        base["b2"] = _pack_bias(b2)
    if with_b1 or with_b2:
        base["ind"] = _make_ind()

    in_maps = []
    for i in range(NCORES):
        m = dict(base)
        m["x"] = _pack_x_core(x[i * BL : (i + 1) * BL].astype(np.float32), t_steps)
        in_maps.append(m)

    res = run_bass_kernel_spmd(nc, in_maps, list(range(NCORES)), trace=TRACE)
    LAST_EXEC_NS = res.exec_time_ns

    h2 = np.concatenate(
        [
            res.results[i]["y"].reshape(128, 2, 64).transpose(2, 1, 0).reshape(64, 256)
            for i in range(NCORES)
        ],
        axis=0,
    )
    return (h2.astype(np.float32) @ Wout + bout).astype(np.float32)
